# revision 14
# baseline (speedup 1.0000x reference)
"""Transformer block (pre-LN, non-causal full softmax, no 1/sqrt(D) scaling)
on 8 TRN2 NeuronCores.

Sharding: core c owns batch b = c//2 and query-token half q = c%2. The host
rolls each batch's token axis so the core's query half is always rows 0:512
(full non-causal attention is permutation-invariant over key/value tokens,
so rolling the kv axis changes nothing). K/V are computed over the full
1024 tokens of the batch on both cores that share it — duplicated KV
compute instead of any cross-core communication.

On-chip dataflow keeps activations feature-major ("h^T" = [E, tokens]) so
every matmul is lhsT.T @ rhs with the contraction on partitions. Matmuls
run in float32r (full-rate fp32); V and the exp(scores) tiles are bf16
(post-softmax linear averaging, errors cancel). HW constraints found
empirically: fp32r matmul operands/psum outputs must sit at partition
base 0, so odd heads' K^T/Q^T strips are staged to base-0 tiles with
SBUF->SBUF DMA partition shifts just-in-time.
"""

import ml_dtypes
import numpy as np

import concourse.bass as bass
import concourse.mybir as mybir
import concourse.tile as tile
from concourse import bacc
from concourse.bass_utils import run_bass_kernel_spmd

F32 = mybir.dt.float32
F32R = mybir.dt.float32r
BF16 = mybir.dt.bfloat16
AF = mybir.ActivationFunctionType
ALU = mybir.AluOpType

B, T, E, H, D, FF = 4, 1024, 1024, 16, 64, 4096
TQ = 512
NCORES = 8
EPS = 1e-5
P = 128

_CACHE: dict = {}


def _emit(nc, tc, d, out_d):
    const_cm = tc.tile_pool(name="const", bufs=1, side="right")
    const = const_cm.__enter__()
    eye = const.tile([P, P], BF16)
    ones = const.tile([P, P], F32R)
    ones_bf = const.tile([P, P], BF16)
    nc.sync.dma_start(out=eye[:], in_=d["eye"][:, :])
    nc.sync.dma_start(out=ones[:], in_=d["ones"][:, :])
    nc.sync.dma_start(out=ones_bf[:], in_=d["ones_bf"][:, :])
    epst = const.tile([P, 1], F32)
    nc.vector.memset(epst[:], EPS)
    bias = {}
    for name, w in [("ln1g", 8), ("ln1b", 8), ("ln2g", 8), ("ln2b", 8),
                    ("bp", 8), ("b1", 32), ("b2", 8)]:
        bias[name] = const.tile([P, w], F32, tag=f"bias_{name}", name=f"bias_{name}")
        nc.sync.dma_start(out=bias[name][:], in_=d[name][:, :])

    # long-lived activation pools: opened just before first use (pool space
    # is reserved at open), closed right after last use; lifetimes overlap
    # non-hierarchically so they are managed manually.
    hT_cm = tc.tile_pool(name="hTp", side="left", bufs=1)
    hTp = hT_cm.__enter__()
    hT = [hTp.tile([P, T], BF16, tag=f"hT{j}", name=f"hT{j}") for j in range(8)]

    # ---- stage 1: LN1 + transpose to feature-major h^T ----
    with tc.tile_pool(name="s1x", side="left", bufs=3) as xp, \
         tc.tile_pool(name="s1s", side="left", bufs=4) as sp, \
         tc.tile_pool(name="s1ps", bufs=4, space="PSUM") as tpp:
        for i in range(8):
            xt = xp.tile([P, E], F32, tag="xt")
            nc.sync.dma_start(out=xt[:], in_=d["x"][i * P:(i + 1) * P, :])
            stats = sp.tile([P, 2, 6], F32, tag="stats")
            nc.vector.bn_stats(stats[:, 0, :], xt[:, 0:512])
            nc.vector.bn_stats(stats[:, 1, :], xt[:, 512:1024])
            mv = sp.tile([P, 2], F32, tag="mv")
            nc.vector.bn_aggr(mv[:], stats[:])
            rsig = sp.tile([P, 1], F32, tag="rsig")
            nc.scalar.activation(rsig[:], mv[:, 1:2], AF.Sqrt, bias=epst[:])
            nc.vector.reciprocal(rsig[:], rsig[:])
            xn = xp.tile([P, E], BF16, tag="xn")
            nc.vector.tensor_scalar(xn[:], xt[:], mv[:, 0:1], rsig[:],
                                    ALU.subtract, op1=ALU.mult)
            for j in range(8):
                pt = tpp.tile([P, P], BF16, tag="tp")
                nc.tensor.transpose(pt[:], xn[:, j * P:(j + 1) * P], eye[:])
                nc.scalar.activation(hT[j][:, i * P:(i + 1) * P], pt[:],
                                     AF.Identity, bias=bias["ln1b"][:, j:j + 1],
                                     scale=bias["ln1g"][:, j:j + 1])

    # ---- stage 2: projections K^T, V, Q^T (chunk-major storage) ----
    kv_cm = tc.tile_pool(name="kvp", bufs=1, side="right")
    kvp = kv_cm.__enter__()
    ktc = [kvp.tile([P, T], F32R, tag=f"kt{m}", name=f"kt{m}") for m in range(8)]
    qtc = [kvp.tile([P, TQ], F32R, tag=f"qt{m}", name=f"qt{m}") for m in range(8)]
    vv = [kvp.tile([P, E], BF16, tag=f"v{i}", name=f"v{i}") for i in range(8)]

    def load_w(pool, wname):
        ws = []
        for k in range(8):
            w = pool.tile([P, E], BF16, tag=f"w{k}")
            nc.sync.dma_start(out=w[:], in_=d[wname][k * P:(k + 1) * P, :])
            ws.append(w)
        return ws

    with tc.tile_pool(name="wk_s", side="left", bufs=1) as wp_, \
         tc.tile_pool(name="ps_k", bufs=4, space="PSUM") as pp_:
        ws = load_w(wp_, "wk")
        for m in range(8):
            for n in range(2):
                ps = pp_.tile([P, 512], F32, tag="ps")
                for k in range(8):
                    nc.tensor.matmul(ps[:], ws[k][:, m * P:(m + 1) * P],
                                     hT[k][:, n * 512:(n + 1) * 512],
                                     start=(k == 0), stop=(k == 7))
                nc.scalar.copy(ktc[m][:, n * 512:(n + 1) * 512], ps[:])

    with tc.tile_pool(name="wv_s", side="left", bufs=1) as wp_, \
         tc.tile_pool(name="ps_v", bufs=4, space="PSUM") as pp_:
        ws = load_w(wp_, "wv")
        for i in range(8):
            for n in range(2):
                ps = pp_.tile([P, 512], F32, tag="ps")
                for k in range(8):
                    nc.tensor.matmul(ps[:], hT[k][:, i * P:(i + 1) * P],
                                     ws[k][:, n * 512:(n + 1) * 512],
                                     start=(k == 0), stop=(k == 7))
                nc.scalar.copy(vv[i][:, n * 512:(n + 1) * 512], ps[:])

    with tc.tile_pool(name="wq_s", side="left", bufs=1) as wp_, \
         tc.tile_pool(name="ps_q", bufs=4, space="PSUM") as pp_:
        ws = load_w(wp_, "wq")
        for m in range(8):
            ps = pp_.tile([P, 512], F32, tag="ps")
            for k in range(8):
                nc.tensor.matmul(ps[:], ws[k][:, m * P:(m + 1) * P],
                                 hT[k][:, 0:512], start=(k == 0), stop=(k == 7))
            nc.scalar.copy(qtc[m][:], ps[:])

    hT_cm.__exit__(None, None, None)

    # ---- stage 3: attention per head ----
    ot_cm = tc.tile_pool(name="otp", side="left", bufs=1)
    otp = ot_cm.__enter__()
    ot = [otp.tile([D, TQ], BF16, tag=f"ot{h}", name=f"ot{h}") for h in range(H)]
    with tc.tile_pool(name="att_pt", side="left", bufs=18) as ptp, \
         tc.tile_pool(name="att_sh", side="left", bufs=2) as shp, \
         tc.tile_pool(name="att_sc", side="left", bufs=4) as scp, \
         tc.tile_pool(name="ps_s", bufs=4, space="PSUM") as spp, \
         tc.tile_pool(name="ps_o", bufs=2, space="PSUM") as opp, \
         tc.tile_pool(name="ps_m", bufs=1, space="PSUM") as mpp:
        # software-pipelined over heads: scores/exp for head h are issued
        # before the PV/normalize of head h-1 so PE stays busy while ACT
        # works through the exps (keeps the HAM clock-gate warm).
        state = {}

        def s_phase(h):
            m = h // 2
            if h % 2 == 0:
                kth = ktc[m][0:D, :]
                qth = qtc[m][0:D, :]
            else:
                ksh = shp.tile([D, T], F32R, tag="ksh", name="ksh")
                nc.sync.dma_start(out=ksh[0:D, :], in_=ktc[m][D:P, :])
                qsh = shp.tile([D, TQ], F32R, tag="qsh", name="qsh")
                nc.sync.dma_start(out=qsh[0:D, :], in_=qtc[m][D:P, :])
                kth = ksh[0:D, :]
                qth = qsh[0:D, :]
            ptj = []
            for j in range(8):
                ps_s = spp.tile([P, 512], F32, tag="ps_s", name="ps_s")
                nc.tensor.matmul(ps_s[:], kth[:, j * P:(j + 1) * P], qth,
                                 start=True, stop=True)
                pt_ = ptp.tile([P, 512], BF16, tag="pt", name="pt_")
                nc.scalar.activation(pt_[:], ps_s[:], AF.Exp)
                ptj.append(pt_)
            state[h] = ptj

        def pv_phase(h):
            ptj = state.pop(h)
            ps_o = opp.tile([D, 512], F32, tag="ps_o", name="ps_o")
            for j in range(8):
                nc.tensor.matmul(ps_o[0:D, :], vv[j][:, h * D:(h + 1) * D],
                                 ptj[j][:], start=(j == 0), stop=(j == 7))
            ps_m = mpp.tile([1, 512], F32, tag="ps_m", name="ps_m")
            for j in range(8):
                nc.tensor.matmul(ps_m[0:1, :], ones_bf[:, 0:1], ptj[j][:],
                                 start=(j == 0), stop=(j == 7))
            sums = scp.tile([1, 512], F32R, tag="sums", name="sums")
            nc.scalar.copy(sums[0:1, :], ps_m[0:1, :])
            ps_b = mpp.tile([D, 512], F32, tag="ps_b", name="ps_b")
            nc.tensor.matmul(ps_b[0:D, :], ones[0:1, 0:D], sums[0:1, :],
                             start=True, stop=True)
            bc = scp.tile([D, 512], F32, tag="bc", name="bc")
            nc.vector.tensor_copy(bc[0:D, :], ps_b[0:D, :])
            bcr = scp.tile([D, 512], F32, tag="bcr", name="bcr")
            nc.vector.reciprocal_approx_fast(bcr[0:D, :], bc[0:D, :])
            nc.vector.scalar_tensor_tensor(ot[h][0:D, :], ps_o[0:D, :], 1.0,
                                           bcr[0:D, :], ALU.mult, ALU.mult)

        for h in range(H + 1):
            if h >= 1:
                pv_phase(h - 1)
            if h < H:
                s_phase(h)

    kv_cm.__exit__(None, None, None)

    # ---- stage 4: attn out proj (+bias) -> residual -> LN2 ----
    x2s_cm = tc.tile_pool(name="x2s", bufs=1, side="right")
    x2s = x2s_cm.__enter__()
    x2 = [x2s.tile([P, E], F32, tag=f"x2_{i}", name=f"x2_{i}") for i in range(4)]
    h2T = [x2s.tile([P, TQ], BF16, tag=f"h2T{j}", name=f"h2T{j}") for j in range(8)]
    aot = [x2s.tile([P, TQ], BF16, tag=f"aot{m}", name=f"aot{m}") for m in range(8)]
    with tc.tile_pool(name="wp_s", side="left", bufs=4) as wpp, \
         tc.tile_pool(name="ps_p", bufs=1, space="PSUM") as ppp:
        psm = [ppp.tile([P, TQ], F32, tag=f"pp{m}", name=f"pp{m}") for m in range(8)]
        for h in range(H):
            w = wpp.tile([D, E], BF16, tag="wp")
            nc.sync.dma_start(out=w[0:D, :], in_=d["wp"][h * D:(h + 1) * D, :])
            for m in range(8):
                nc.tensor.matmul(psm[m][:], w[0:D, m * P:(m + 1) * P],
                                 ot[h][0:D, :], start=(h == 0), stop=(h == H - 1))
        for m in range(8):
            nc.scalar.activation(aot[m][:], psm[m][:], AF.Identity,
                                 bias=bias["bp"][:, m:m + 1])

    ot_cm.__exit__(None, None, None)

    with tc.tile_pool(name="s4x", side="left", bufs=2) as xqp, \
         tc.tile_pool(name="s4s", side="left", bufs=4) as sp, \
         tc.tile_pool(name="s4ps", bufs=4, space="PSUM") as tpp:
        for i in range(4):
            xq = xqp.tile([P, E], F32, tag="xq")
            nc.sync.dma_start(out=xq[:], in_=d["x"][i * P:(i + 1) * P, :])
            for j in range(8):
                pt = tpp.tile([P, P], BF16, tag="tp")
                nc.tensor.transpose(pt[:], aot[j][:, i * P:(i + 1) * P], eye[:])
                nc.vector.scalar_tensor_tensor(
                    x2[i][:, j * P:(j + 1) * P], pt[:], 1.0,
                    xq[:, j * P:(j + 1) * P], ALU.mult, ALU.add)
            stats = sp.tile([P, 2, 6], F32, tag="stats")
            nc.vector.bn_stats(stats[:, 0, :], x2[i][:, 0:512])
            nc.vector.bn_stats(stats[:, 1, :], x2[i][:, 512:1024])
            mv = sp.tile([P, 2], F32, tag="mv")
            nc.vector.bn_aggr(mv[:], stats[:])
            rsig = sp.tile([P, 1], F32, tag="rsig")
            nc.scalar.activation(rsig[:], mv[:, 1:2], AF.Sqrt, bias=epst[:])
            nc.vector.reciprocal(rsig[:], rsig[:])
            xn = sp.tile([P, E], BF16, tag="xn")
            nc.vector.tensor_scalar(xn[:], x2[i][:], mv[:, 0:1], rsig[:],
                                    ALU.subtract, op1=ALU.mult)
            for j in range(8):
                pt = tpp.tile([P, P], BF16, tag="tp")
                nc.tensor.transpose(pt[:], xn[:, j * P:(j + 1) * P], eye[:])
                nc.scalar.activation(h2T[j][:, i * P:(i + 1) * P], pt[:],
                                     AF.Identity, bias=bias["ln2b"][:, j:j + 1],
                                     scale=bias["ln2g"][:, j:j + 1])

    # ---- stage 5: FFN ----
    rr_cm = tc.tile_pool(name="relu", side="left", bufs=1)
    rrp = rr_cm.__enter__()
    rr = [rrp.tile([P, TQ], BF16, tag=f"r{k}", name=f"r{k}") for k in range(32)]
    with tc.tile_pool(name="w1s", side="left", bufs=1) as w1p, \
         tc.tile_pool(name="ps_f1", bufs=4, space="PSUM") as fpp:
        for g in range(4):
            ws = []
            for k in range(8):
                w = w1p.tile([P, 1024], BF16, tag=f"w1_{k}")
                nc.sync.dma_start(
                    out=w[:], in_=d["w1"][k * P:(k + 1) * P,
                                          g * 1024:(g + 1) * 1024])
                ws.append(w)
            for m in range(8):
                ps = fpp.tile([P, TQ], F32, tag="ps")
                for k in range(8):
                    nc.tensor.matmul(ps[:], ws[k][:, m * P:(m + 1) * P],
                                     h2T[k][:], start=(k == 0), stop=(k == 7))
                col = g * 8 + m
                nc.scalar.activation(rr[col][:], ps[:], AF.Relu,
                                     bias=bias["b1"][:, col:col + 1])
    f2t = [x2s.tile([P, TQ], BF16, tag=f"f2t{m}", name=f"f2t{m}") for m in range(8)]
    with tc.tile_pool(name="w2s", side="left", bufs=3) as w2p, \
         tc.tile_pool(name="ps_f2", bufs=1, space="PSUM") as fpp:
        psm = [fpp.tile([P, TQ], F32, tag=f"pf{m}", name=f"pf{m}") for m in range(8)]
        for k in range(32):
            w = w2p.tile([P, E], BF16, tag="w2")
            nc.sync.dma_start(out=w[:], in_=d["w2"][k * P:(k + 1) * P, :])
            for m in range(8):
                nc.tensor.matmul(psm[m][:], w[:, m * P:(m + 1) * P], rr[k][:],
                                 start=(k == 0), stop=(k == 31))
        for m in range(8):
            nc.scalar.activation(f2t[m][:], psm[m][:], AF.Identity,
                                 bias=bias["b2"][:, m:m + 1])
    rr_cm.__exit__(None, None, None)

    with tc.tile_pool(name="outp", side="left", bufs=2) as outp, \
         tc.tile_pool(name="ps_t5", bufs=4, space="PSUM") as tpp:
        for i in range(4):
            out_i = outp.tile([P, E], F32, tag="out")
            for j in range(8):
                pt = tpp.tile([P, P], BF16, tag="tp")
                nc.tensor.transpose(pt[:], f2t[j][:, i * P:(i + 1) * P], eye[:])
                nc.vector.scalar_tensor_tensor(
                    out_i[:, j * P:(j + 1) * P], pt[:], 1.0,
                    x2[i][:, j * P:(j + 1) * P], ALU.mult, ALU.add)
            nc.sync.dma_start(out=out_d[i * P:(i + 1) * P, :], in_=out_i[:])

    x2s_cm.__exit__(None, None, None)
    const_cm.__exit__(None, None, None)


def _build():
    nc = bacc.Bacc("TRN2", target_bir_lowering=False, debug=False)
    d = {}

    def din(name, shape, dt=F32R):
        d[name] = nc.dram_tensor(name, shape, dt, kind="ExternalInput").ap()

    din("x", [T, E], F32)
    for n in ("wq", "wk", "wv"):
        din(n, [E, E], BF16)
    din("wp", [E, E], BF16)
    din("w1", [E, FF], BF16)
    din("w2", [FF, E], BF16)
    din("eye", [P, P], BF16)
    din("ones", [P, P])
    din("ones_bf", [P, P], BF16)
    for n, w in [("ln1g", 8), ("ln1b", 8), ("ln2g", 8), ("ln2b", 8),
                 ("bp", 8), ("b1", 32), ("b2", 8)]:
        din(n, [P, w], F32)
    out_d = nc.dram_tensor("out", [TQ, E], F32, kind="ExternalOutput").ap()
    with nc.allow_low_precision(reason="fp32r compute"):
        with tile.TileContext(nc) as tc:
            _emit(nc, tc, d, out_d)
    nc.compile()
    return nc


def _get_nc():
    if "nc" not in _CACHE:
        _CACHE["nc"] = _build()
    return _CACHE["nc"]


def _colmajor_bias(v, width):
    return np.ascontiguousarray(np.asarray(v, np.float32).reshape(width, P).T)


def make_in_maps(x, ln1_g, ln1_b, Wq, Wk, Wv, Wp, bp, ln2_g, ln2_b,
                 W1, b1, W2, b2):
    x = np.asarray(x, dtype=np.float32)
    shared = {
        "wq": np.ascontiguousarray(
            np.transpose(np.asarray(Wq, np.float32), (1, 0, 2)).reshape(E, E)
        ).astype(ml_dtypes.bfloat16),
        "wk": np.ascontiguousarray(
            np.transpose(np.asarray(Wk, np.float32), (1, 0, 2)).reshape(E, E)
        ).astype(ml_dtypes.bfloat16),
        "wv": np.ascontiguousarray(
            np.transpose(np.asarray(Wv, np.float32), (1, 0, 2)).reshape(E, E)
        ).astype(ml_dtypes.bfloat16),
        "wp": np.asarray(Wp, np.float32).astype(ml_dtypes.bfloat16),
        "w1": np.asarray(W1, np.float32).astype(ml_dtypes.bfloat16),
        "w2": np.asarray(W2, np.float32).astype(ml_dtypes.bfloat16),
        "eye": np.eye(P, dtype=ml_dtypes.bfloat16),
        "ones": np.ones((P, P), dtype=np.float32),
        "ones_bf": np.ones((P, P), dtype=ml_dtypes.bfloat16),
        "ln1g": _colmajor_bias(ln1_g, 8),
        "ln1b": _colmajor_bias(ln1_b, 8),
        "ln2g": _colmajor_bias(ln2_g, 8),
        "ln2b": _colmajor_bias(ln2_b, 8),
        "bp": _colmajor_bias(bp, 8),
        "b1": _colmajor_bias(b1, 32),
        "b2": _colmajor_bias(b2, 8),
    }
    in_maps = []
    for c in range(NCORES):
        b = c // 2
        q0 = TQ * (c % 2)
        xb = x[b]
        x_roll = np.ascontiguousarray(np.concatenate([xb[q0:], xb[:q0]], axis=0))
        in_maps.append({"x": x_roll, **shared})
    return in_maps


def assemble_out(results):
    out = np.empty((B, T, E), dtype=np.float32)
    for c in range(NCORES):
        b = c // 2
        q0 = TQ * (c % 2)
        out[b, q0:q0 + TQ] = results[c]["out"]
    return out


def kernel(x, ln1_g, ln1_b, Wq, Wk, Wv, Wp, bp, ln2_g, ln2_b, W1, b1, W2, b2,
           **_ignored):
    in_maps = make_in_maps(x, ln1_g, ln1_b, Wq, Wk, Wv, Wp, bp,
                           ln2_g, ln2_b, W1, b1, W2, b2)
    nc = _get_nc()
    res = run_bass_kernel_spmd(nc, in_maps, core_ids=list(range(NCORES)))
    return assemble_out(res.results)


# revision 20
# speedup vs baseline: 1.3157x; 1.3157x over previous
"""Transformer block (pre-LN, non-causal full softmax, no 1/sqrt(D) scaling)
on 8 TRN2 NeuronCores.

Sharding: core c owns batch b = c//2 and query-token half q = c%2. The host
rolls each batch's token axis so the core's query half is always rows 0:512
(full non-causal attention is permutation-invariant over key/value tokens,
so rolling the kv axis changes nothing). K/V are computed over the full
1024 tokens of the batch on both cores that share it — duplicated KV
compute instead of any cross-core communication.

On-chip dataflow keeps activations feature-major ("h^T" = [E, tokens]) so
every matmul is lhsT.T @ rhs with the contraction on partitions. Matmuls
run in float32r (full-rate fp32); V and the exp(scores) tiles are bf16
(post-softmax linear averaging, errors cancel). HW constraints found
empirically: fp32r matmul operands/psum outputs must sit at partition
base 0, so odd heads' K^T/Q^T strips are staged to base-0 tiles with
SBUF->SBUF DMA partition shifts just-in-time.
"""

import ml_dtypes
import numpy as np

import concourse.bass as bass
import concourse.mybir as mybir
import concourse.tile as tile
from concourse import bacc
from concourse.bass_utils import run_bass_kernel_spmd

F32 = mybir.dt.float32
F32R = mybir.dt.float32r
BF16 = mybir.dt.bfloat16
AF = mybir.ActivationFunctionType
ALU = mybir.AluOpType

B, T, E, H, D, FF = 4, 1024, 1024, 16, 64, 4096
TQ = 512
NCORES = 8
EPS = 1e-5
P = 128

_CACHE: dict = {}


def _emit(nc, tc, d, out_d):
    const_cm = tc.tile_pool(name="const", bufs=1, side="right")
    const = const_cm.__enter__()
    eye = const.tile([P, P], BF16)
    ones_bf = const.tile([P, P], BF16)
    selb = const.tile([33, P], F32R)
    nc.sync.dma_start(out=eye[:], in_=d["eye"][:, :])
    nc.sync.dma_start(out=ones_bf[:], in_=d["ones_bf"][:, :])
    nc.sync.dma_start(out=selb[:], in_=d["selb"][:, :])
    epst = const.tile([P, 1], F32)
    nc.vector.memset(epst[:], EPS)
    bias = {}
    for name, w in [("ln1g", 8), ("ln1b", 8), ("ln2g", 8), ("ln2b", 8),
                    ("bp", 8), ("b1", 32), ("b2", 8)]:
        bias[name] = const.tile([P, w], F32, tag=f"bias_{name}", name=f"bias_{name}")
        nc.sync.dma_start(out=bias[name][:], in_=d[name][:, :])

    # long-lived activation pools: opened just before first use (pool space
    # is reserved at open), closed right after last use; lifetimes overlap
    # non-hierarchically so they are managed manually.
    hT_cm = tc.tile_pool(name="hTp", side="left", bufs=1)
    hTp = hT_cm.__enter__()
    hT = [hTp.tile([P, T], BF16, tag=f"hT{j}", name=f"hT{j}") for j in range(8)]

    # ---- stage 1: LN1 + transpose to feature-major h^T ----
    with tc.tile_pool(name="s1x", side="left", bufs=3) as xp, \
         tc.tile_pool(name="s1s", side="left", bufs=4) as sp, \
         tc.tile_pool(name="s1ps", bufs=4, space="PSUM") as tpp:
        for i in range(8):
            xt = xp.tile([P, E], F32, tag="xt")
            nc.sync.dma_start(out=xt[:], in_=d["x"][i * P:(i + 1) * P, :])
            stats = sp.tile([P, 2, 6], F32, tag="stats")
            nc.vector.bn_stats(stats[:, 0, :], xt[:, 0:512])
            nc.vector.bn_stats(stats[:, 1, :], xt[:, 512:1024])
            mv = sp.tile([P, 2], F32, tag="mv")
            nc.vector.bn_aggr(mv[:], stats[:])
            rsig = sp.tile([P, 1], F32, tag="rsig")
            nc.scalar.activation(rsig[:], mv[:, 1:2], AF.Sqrt, bias=epst[:])
            nc.vector.reciprocal(rsig[:], rsig[:])
            xn = xp.tile([P, E], BF16, tag="xn")
            nc.vector.tensor_scalar(xn[:], xt[:], mv[:, 0:1], rsig[:],
                                    ALU.subtract, op1=ALU.mult)
            for j in range(8):
                pt = tpp.tile([P, P], BF16, tag="tp")
                nc.tensor.transpose(pt[:], xn[:, j * P:(j + 1) * P], eye[:])
                nc.scalar.activation(hT[j][:, i * P:(i + 1) * P], pt[:],
                                     AF.Identity, bias=bias["ln1b"][:, j:j + 1],
                                     scale=bias["ln1g"][:, j:j + 1])

    # ---- stage 2: projections K^T, V, Q^T (chunk-major storage) ----
    kv_cm = tc.tile_pool(name="kvp", bufs=1, side="right")
    kvp = kv_cm.__enter__()
    ktc = [kvp.tile([P, T], BF16, tag=f"kt{m}", name=f"kt{m}") for m in range(8)]
    qtc = [kvp.tile([P, TQ], BF16, tag=f"qt{m}", name=f"qt{m}") for m in range(8)]
    vv = [kvp.tile([P, E], BF16, tag=f"v{i}", name=f"v{i}") for i in range(8)]

    def load_w(pool, wname):
        ws = []
        for k in range(8):
            w = pool.tile([P, E], BF16, tag=f"w{k}")
            nc.sync.dma_start(out=w[:], in_=d[wname][k * P:(k + 1) * P, :])
            ws.append(w)
        return ws

    with tc.tile_pool(name="wk_s", side="left", bufs=1) as wp_, \
         tc.tile_pool(name="ps_k", bufs=4, space="PSUM") as pp_:
        ws = load_w(wp_, "wk")
        for m in range(8):
            for n in range(2):
                ps = pp_.tile([P, 512], F32, tag="ps")
                for k in range(8):
                    nc.tensor.matmul(ps[:], ws[k][:, m * P:(m + 1) * P],
                                     hT[k][:, n * 512:(n + 1) * 512],
                                     start=(k == 0), stop=(k == 7))
                nc.scalar.copy(ktc[m][:, n * 512:(n + 1) * 512], ps[:])

    with tc.tile_pool(name="wv_s", side="left", bufs=1) as wp_, \
         tc.tile_pool(name="ps_v", bufs=4, space="PSUM") as pp_:
        ws = load_w(wp_, "wv")
        for i in range(8):
            for n in range(2):
                ps = pp_.tile([P, 512], F32, tag="ps")
                for k in range(8):
                    nc.tensor.matmul(ps[:], hT[k][:, i * P:(i + 1) * P],
                                     ws[k][:, n * 512:(n + 1) * 512],
                                     start=(k == 0), stop=(k == 7))
                nc.scalar.copy(vv[i][:, n * 512:(n + 1) * 512], ps[:])

    with tc.tile_pool(name="wq_s", side="left", bufs=1) as wp_, \
         tc.tile_pool(name="ps_q", bufs=4, space="PSUM") as pp_:
        ws = load_w(wp_, "wq")
        for m in range(8):
            ps = pp_.tile([P, 512], F32, tag="ps")
            for k in range(8):
                nc.tensor.matmul(ps[:], ws[k][:, m * P:(m + 1) * P],
                                 hT[k][:, 0:512], start=(k == 0), stop=(k == 7))
            nc.scalar.copy(qtc[m][:], ps[:])

    hT_cm.__exit__(None, None, None)

    # ---- stage 3: attention, one head-pair at a time ----
    # Pair p = heads (2p, 2p+1) living in chunk tiles ktc[p]/qtc[p] rows
    # [0:64] / [64:128]. Scores run row-packed (two K=64 matmuls in disjoint
    # row groups), PV runs col-packed (two M=64 matmuls into the top/bottom
    # halves of one psum tile) accumulating chunk-major O^T. The attn-out
    # projection (K=128 per chunk, FWL-eligible weights) follows inside the
    # same pool scope, reusing the score psum banks.
    ot_cm = tc.tile_pool(name="otp", side="left", bufs=1)
    otp = ot_cm.__enter__()
    otc = [otp.tile([P, TQ], BF16, tag=f"ot{p}", name=f"ot{p}") for p in range(8)]
    wps = [otp.tile([P, E], BF16, tag=f"wp{p}", name=f"wp{p}") for p in range(8)]
    aot = []
    with tc.tile_pool(name="att_pt", side="left", bufs=18) as ptp, \
         tc.tile_pool(name="att_sc", side="left", bufs=4) as scp, \
         tc.tile_pool(name="ps_s", bufs=4, space="PSUM") as spp, \
         tc.tile_pool(name="ps_o", bufs=2, space="PSUM") as opp, \
         tc.tile_pool(name="ps_m", bufs=1, space="PSUM") as mpp:
        state = {}

        def s_phase(p):
            ptj = []
            for j in range(8):
                for half in range(2):
                    ps_s = spp.tile([P, 512], F32, tag="ps_s", name="ps_s")
                    nc.tensor.matmul(ps_s[:],
                                     ktc[p][64 * half:64 * half + 64,
                                            j * P:(j + 1) * P],
                                     qtc[p][64 * half:64 * half + 64, :],
                                     start=True, stop=True)
                    pt_ = ptp.tile([P, 512], BF16, tag="pt", name="pt_")
                    nc.scalar.activation(pt_[:], ps_s[:], AF.Exp)
                    ptj.append(pt_)
            state[p] = ptj

        def pv_phase(p):
            ptj = state.pop(p)
            ps_pair = opp.tile([P, 512], F32, tag="ps_o", name="ps_pair")
            for j in range(8):
                nc.tensor.matmul(ps_pair[0:64, :],
                                 vv[j][:, (2 * p) * D:(2 * p) * D + D],
                                 ptj[2 * j][:], start=(j == 0), stop=(j == 7),
                                 skip_group_check=True)
                nc.tensor.matmul(ps_pair[64:128, :],
                                 vv[j][:, (2 * p + 1) * D:(2 * p + 1) * D + D],
                                 ptj[2 * j + 1][:], start=(j == 0), stop=(j == 7),
                                 skip_group_check=True)
            ps_sm = mpp.tile([33, 512], F32, tag="ps_sm", name="ps_sm")
            for j in range(8):
                nc.tensor.matmul(ps_sm[0:1, :], ones_bf[:, 0:1], ptj[2 * j][:],
                                 start=(j == 0), stop=(j == 7),
                                 skip_group_check=True)
                nc.tensor.matmul(ps_sm[32:33, :], ones_bf[:, 0:1],
                                 ptj[2 * j + 1][:], start=(j == 0), stop=(j == 7),
                                 tile_position=(0, 32), skip_group_check=True)
            sums = scp.tile([33, 512], F32R, tag="sums", name="sums")
            nc.gpsimd.memset(sums[0:33, :].bitcast(F32), 1.0)
            nc.vector.tensor_copy(sums[0:1, :], ps_sm[0:1, :])
            nc.vector.tensor_copy(sums[32:33, :], ps_sm[32:33, :])
            ps_b = mpp.tile([P, 512], F32, tag="ps_b", name="ps_b")
            nc.tensor.matmul(ps_b[:], selb[0:33, :], sums[0:33, :],
                             start=True, stop=True)
            bc = scp.tile([P, 512], F32, tag="bc", name="bc")
            nc.vector.tensor_copy(bc[:], ps_b[:])
            bcr = scp.tile([P, 512], F32, tag="bcr", name="bcr")
            nc.vector.reciprocal_approx_fast(bcr[:], bc[:])
            nc.vector.scalar_tensor_tensor(otc[p][:], ps_pair[:], 1.0,
                                           bcr[:], ALU.mult, ALU.mult)

        for p in range(8):
            nc.sync.dma_start(out=wps[p][:], in_=d["wp"][p * P:(p + 1) * P, :])
            if p >= 1:
                pv_phase(p - 1)
            s_phase(p)
        pv_phase(7)

        # attn-out projection: 8 m-chunks, accumulate over the 8 p-chunks,
        # reusing the (now idle) score psum banks 4 at a time.
        for mg in range(2):
            pj = []
            for mm in range(4):
                m = mg * 4 + mm
                ps = spp.tile([P, TQ], F32, tag="ps_s", name="ps")
                for p in range(8):
                    nc.tensor.matmul(ps[:], wps[p][:, m * P:(m + 1) * P],
                                     otc[p][:], start=(p == 0), stop=(p == 7))
                pj.append((m, ps))
            for m, ps in pj:
                a = otp.tile([P, TQ], BF16, tag=f"aot{m}", name=f"aot{m}")
                nc.scalar.activation(a[:], ps[:], AF.Identity,
                                     bias=bias["bp"][:, m:m + 1])
                aot.append(a)

    kv_cm.__exit__(None, None, None)

    # ---- stage 4: residual -> LN2 ----
    x2s_cm = tc.tile_pool(name="x2s", bufs=1, side="right")
    x2s = x2s_cm.__enter__()
    x2 = [x2s.tile([P, E], F32, tag=f"x2_{i}", name=f"x2_{i}") for i in range(4)]
    h2T = [x2s.tile([P, TQ], BF16, tag=f"h2T{j}", name=f"h2T{j}") for j in range(8)]

    with tc.tile_pool(name="s4x", side="left", bufs=2) as xqp, \
         tc.tile_pool(name="s4s", side="left", bufs=4) as sp, \
         tc.tile_pool(name="s4ps", bufs=4, space="PSUM") as tpp:
        for i in range(4):
            xq = xqp.tile([P, E], F32, tag="xq")
            nc.sync.dma_start(out=xq[:], in_=d["x"][i * P:(i + 1) * P, :])
            for j in range(8):
                pt = tpp.tile([P, P], BF16, tag="tp")
                nc.tensor.transpose(pt[:], aot[j][:, i * P:(i + 1) * P], eye[:])
                nc.vector.scalar_tensor_tensor(
                    x2[i][:, j * P:(j + 1) * P], pt[:], 1.0,
                    xq[:, j * P:(j + 1) * P], ALU.mult, ALU.add)
            stats = sp.tile([P, 2, 6], F32, tag="stats")
            nc.vector.bn_stats(stats[:, 0, :], x2[i][:, 0:512])
            nc.vector.bn_stats(stats[:, 1, :], x2[i][:, 512:1024])
            mv = sp.tile([P, 2], F32, tag="mv")
            nc.vector.bn_aggr(mv[:], stats[:])
            rsig = sp.tile([P, 1], F32, tag="rsig")
            nc.scalar.activation(rsig[:], mv[:, 1:2], AF.Sqrt, bias=epst[:])
            nc.vector.reciprocal(rsig[:], rsig[:])
            xn = sp.tile([P, E], BF16, tag="xn")
            nc.vector.tensor_scalar(xn[:], x2[i][:], mv[:, 0:1], rsig[:],
                                    ALU.subtract, op1=ALU.mult)
            for j in range(8):
                pt = tpp.tile([P, P], BF16, tag="tp")
                nc.tensor.transpose(pt[:], xn[:, j * P:(j + 1) * P], eye[:])
                nc.scalar.activation(h2T[j][:, i * P:(i + 1) * P], pt[:],
                                     AF.Identity, bias=bias["ln2b"][:, j:j + 1],
                                     scale=bias["ln2g"][:, j:j + 1])

    ot_cm.__exit__(None, None, None)

    # ---- stage 5: FFN ----
    rr_cm = tc.tile_pool(name="relu", side="left", bufs=1)
    rrp = rr_cm.__enter__()
    rr = [rrp.tile([P, TQ], BF16, tag=f"r{k}", name=f"r{k}") for k in range(32)]
    with tc.tile_pool(name="w1s", side="left", bufs=1) as w1p, \
         tc.tile_pool(name="ps_f1", bufs=4, space="PSUM") as fpp:
        for g in range(4):
            ws = []
            for k in range(8):
                w = w1p.tile([P, 1024], BF16, tag=f"w1_{k}")
                nc.sync.dma_start(
                    out=w[:], in_=d["w1"][k * P:(k + 1) * P,
                                          g * 1024:(g + 1) * 1024])
                ws.append(w)
            for m in range(8):
                ps = fpp.tile([P, TQ], F32, tag="ps")
                for k in range(8):
                    nc.tensor.matmul(ps[:], ws[k][:, m * P:(m + 1) * P],
                                     h2T[k][:], start=(k == 0), stop=(k == 7))
                col = g * 8 + m
                nc.scalar.activation(rr[col][:], ps[:], AF.Relu,
                                     bias=bias["b1"][:, col:col + 1])
    f2t = [x2s.tile([P, TQ], BF16, tag=f"f2t{m}", name=f"f2t{m}") for m in range(8)]
    with tc.tile_pool(name="w2s", side="left", bufs=3) as w2p, \
         tc.tile_pool(name="ps_f2", bufs=1, space="PSUM") as fpp:
        psm = [fpp.tile([P, TQ], F32, tag=f"pf{m}", name=f"pf{m}") for m in range(8)]
        for k in range(32):
            w = w2p.tile([P, E], BF16, tag="w2")
            nc.sync.dma_start(out=w[:], in_=d["w2"][k * P:(k + 1) * P, :])
            for m in range(8):
                nc.tensor.matmul(psm[m][:], w[:, m * P:(m + 1) * P], rr[k][:],
                                 start=(k == 0), stop=(k == 31))
        for m in range(8):
            nc.scalar.activation(f2t[m][:], psm[m][:], AF.Identity,
                                 bias=bias["b2"][:, m:m + 1])
    rr_cm.__exit__(None, None, None)

    with tc.tile_pool(name="outp", side="left", bufs=2) as outp, \
         tc.tile_pool(name="ps_t5", bufs=4, space="PSUM") as tpp:
        for i in range(4):
            out_i = outp.tile([P, E], F32, tag="out")
            for j in range(8):
                pt = tpp.tile([P, P], BF16, tag="tp")
                nc.tensor.transpose(pt[:], f2t[j][:, i * P:(i + 1) * P], eye[:])
                nc.vector.scalar_tensor_tensor(
                    out_i[:, j * P:(j + 1) * P], pt[:], 1.0,
                    x2[i][:, j * P:(j + 1) * P], ALU.mult, ALU.add)
            nc.sync.dma_start(out=out_d[i * P:(i + 1) * P, :], in_=out_i[:])

    x2s_cm.__exit__(None, None, None)
    const_cm.__exit__(None, None, None)


def _build():
    nc = bacc.Bacc("TRN2", target_bir_lowering=False, debug=False)
    d = {}

    def din(name, shape, dt=F32R):
        d[name] = nc.dram_tensor(name, shape, dt, kind="ExternalInput").ap()

    din("x", [T, E], F32)
    for n in ("wq", "wk", "wv"):
        din(n, [E, E], BF16)
    din("wp", [E, E], BF16)
    din("w1", [E, FF], BF16)
    din("w2", [FF, E], BF16)
    din("eye", [P, P], BF16)
    din("ones_bf", [P, P], BF16)
    din("selb", [33, P])
    for n, w in [("ln1g", 8), ("ln1b", 8), ("ln2g", 8), ("ln2b", 8),
                 ("bp", 8), ("b1", 32), ("b2", 8)]:
        din(n, [P, w], F32)
    out_d = nc.dram_tensor("out", [TQ, E], F32, kind="ExternalOutput").ap()
    with nc.allow_low_precision(reason="fp32r compute"):
        with tile.TileContext(nc) as tc:
            _emit(nc, tc, d, out_d)
    nc.compile()
    return nc


def _get_nc():
    if "nc" not in _CACHE:
        _CACHE["nc"] = _build()
    return _CACHE["nc"]


def _selb():
    b = np.zeros((33, P), dtype=np.float32)
    b[0, 0:64] = 1.0
    b[32, 64:128] = 1.0
    return b


def _colmajor_bias(v, width):
    return np.ascontiguousarray(np.asarray(v, np.float32).reshape(width, P).T)


def make_in_maps(x, ln1_g, ln1_b, Wq, Wk, Wv, Wp, bp, ln2_g, ln2_b,
                 W1, b1, W2, b2):
    x = np.asarray(x, dtype=np.float32)
    shared = {
        "wq": np.ascontiguousarray(
            np.transpose(np.asarray(Wq, np.float32), (1, 0, 2)).reshape(E, E)
        ).astype(ml_dtypes.bfloat16),
        "wk": np.ascontiguousarray(
            np.transpose(np.asarray(Wk, np.float32), (1, 0, 2)).reshape(E, E)
        ).astype(ml_dtypes.bfloat16),
        "wv": np.ascontiguousarray(
            np.transpose(np.asarray(Wv, np.float32), (1, 0, 2)).reshape(E, E)
        ).astype(ml_dtypes.bfloat16),
        "wp": np.asarray(Wp, np.float32).astype(ml_dtypes.bfloat16),
        "w1": np.asarray(W1, np.float32).astype(ml_dtypes.bfloat16),
        "w2": np.asarray(W2, np.float32).astype(ml_dtypes.bfloat16),
        "eye": np.eye(P, dtype=ml_dtypes.bfloat16),
        "ones_bf": np.ones((P, P), dtype=ml_dtypes.bfloat16),
        "selb": _selb(),
        "ln1g": _colmajor_bias(ln1_g, 8),
        "ln1b": _colmajor_bias(ln1_b, 8),
        "ln2g": _colmajor_bias(ln2_g, 8),
        "ln2b": _colmajor_bias(ln2_b, 8),
        "bp": _colmajor_bias(bp, 8),
        "b1": _colmajor_bias(b1, 32),
        "b2": _colmajor_bias(b2, 8),
    }
    in_maps = []
    for c in range(NCORES):
        b = c // 2
        q0 = TQ * (c % 2)
        xb = x[b]
        x_roll = np.ascontiguousarray(np.concatenate([xb[q0:], xb[:q0]], axis=0))
        in_maps.append({"x": x_roll, **shared})
    return in_maps


def assemble_out(results):
    out = np.empty((B, T, E), dtype=np.float32)
    for c in range(NCORES):
        b = c // 2
        q0 = TQ * (c % 2)
        out[b, q0:q0 + TQ] = results[c]["out"]
    return out


def kernel(x, ln1_g, ln1_b, Wq, Wk, Wv, Wp, bp, ln2_g, ln2_b, W1, b1, W2, b2,
           **_ignored):
    in_maps = make_in_maps(x, ln1_g, ln1_b, Wq, Wk, Wv, Wp, bp,
                           ln2_g, ln2_b, W1, b1, W2, b2)
    nc = _get_nc()
    res = run_bass_kernel_spmd(nc, in_maps, core_ids=list(range(NCORES)))
    return assemble_out(res.results)
